# revision 1
# baseline (speedup 1.0000x reference)
"""Causal multi-head attention on 8 trn2 NeuronCores.

Problem (hardcoded): x[4, 2048, 768], w_attn[768, 2304], b_attn[2304],
w_proj[768, 768], b_proj[768]; H=6 heads, D=128 head dim; fp32.

Sharding: core c = 2*b + g handles batch b and head-group g (heads
3g..3g+2).  Each core computes Q/K/V projections for its 3 heads over the
full sequence, full causal attention for those heads, and a PARTIAL output
projection (w_proj rows of its heads).  The host sums the two partials per
batch and adds the bias terms.  No cross-core communication.

Bias algebra (host/device split):
  - b_q is added on device (affects scores).
  - b_k is dropped entirely: it shifts every score in a softmax row by the
    same constant, which cancels.
  - b_v is dropped on device: softmax rows sum to 1, so attn @ (v + b_v)
    = attn @ v + b_v; the constant (b_v @ w_proj + b_proj) is added on host.

Device layouts (all transposed, so no on-chip transposes are needed):
  - x is fed as xT [C=768, S=2048];  Q^T/K^T come out as [D, S] per head.
  - scores are computed transposed: sT[kv, rows] = K @ Q^T  (kv on PSUM
    partitions), masked causally, exp'd on the scalar engine straight into
    SBUF.  attn_outT[D, rows] = sum_j V_j^T(lhsT) @ expS_j; the softmax
    denominators come from an all-ones-lhsT matmul accumulated alongside
    (which also replicates them across partitions for the elementwise
    divide).
  - output is written transposed ([768, 2048] partial); host transposes.

Matmuls run as float32r (full fp32 data, reduced-precision PE mode, 1
cycle/row at free-dim >= 256 vs 4 for fp32).  Causal structure is rounded
to 512-row query groups: group t (rows 512t..512t+511) attends to kv
[0, 512(t+1)); the diagonal 4 kv chunks get a host-supplied -1e9 additive
mask.

Scheduling: inputs are split into several DMAs ordered by first use so PE
starts early instead of waiting for the full ~11MB; the attention inner
loop is emitted with a one-batch software-pipeline skew (PE is in-order,
so the PV/rowsum matmuls that wait on exp(batch i) are emitted after the
score matmuls of batch i+1, hiding the scalar-engine latency).
"""

import math
from contextlib import ExitStack

import numpy as np

import concourse.bacc as bacc
import concourse.bass as bass
import concourse.mybir as mybir
import concourse.tile as tile
from concourse import bass_utils

B, S, C = 4, 2048, 768
H, D = 6, 128
HL = 3          # heads per core
CK = C // 128   # 6 contraction chunks
R = 512         # query rows per group
G = S // R      # 4 groups
N_CORES = 8
F32 = mybir.dt.float32
F32R = mybir.dt.float32r
MASK_VAL = -1e9
INV_SQRT_D = 1.0 / math.sqrt(D)
AUXW = HL + 4 * R  # bq columns + 4 mask tiles


def _emit(ctx: ExitStack, tc: tile.TileContext, xa, wav, waqk, aux, ones, wp, outT):
    nc = tc.nc

    singles = ctx.enter_context(tc.tile_pool(name="singles", bufs=1))
    expool = ctx.enter_context(tc.tile_pool(name="expool", bufs=2))
    aopool = ctx.enter_context(tc.tile_pool(name="aopool", bufs=2))
    otpool = ctx.enter_context(tc.tile_pool(name="otpool", bufs=2))
    rspool = ctx.enter_context(tc.tile_pool(name="rspool", bufs=2))
    psum = ctx.enter_context(tc.tile_pool(name="psum", bufs=2, space="PSUM"))

    # ---- resident loads, split + ordered by first use ----
    xa_sb = singles.tile([128, G, CK, R], F32R)     # x, token-chunk major
    wav_sb = singles.tile([128, CK, HL * D], F32R)  # v columns of w_attn
    waqk_sb = singles.tile([128, CK, 2 * HL * D], F32R)
    aux_sb = singles.tile([128, AUXW], F32)         # [bq | 4 causal masks]
    ones_sb = singles.tile([128, 128], F32R)
    wp_sb = singles.tile([128, HL, C], F32R)

    # Ordered by first use: V-projection work (xa chunk 0 + v weights) is the
    # shortest critical prefix, so PE starts ~8us in.
    nc.sync.dma_start(xa_sb[:, 0], xa[:, :CK * R].rearrange("p (c s) -> p c s", c=CK))
    nc.sync.dma_start(wav_sb, wav.rearrange("p (c n) -> p c n", c=CK))
    nc.sync.dma_start(waqk_sb, waqk.rearrange("p (c n) -> p c n", c=CK))
    nc.sync.dma_start(aux_sb, aux)
    nc.sync.dma_start(ones_sb, ones)
    for n in range(1, G):
        nc.sync.dma_start(
            xa_sb[:, n],
            xa[:, n * CK * R:(n + 1) * CK * R].rearrange("p (c s) -> p c s", c=CK))
    nc.sync.dma_start(wp_sb, wp.rearrange("p (f n) -> p f n", f=HL))

    bq_sb = aux_sb[:, 0:HL]

    def mask_ap(k):
        return aux_sb[:, HL + k * R: HL + (k + 1) * R]

    # ---- QKV projections, interleaved per x token-chunk so PE work tracks
    # DMA arrival (xa0, wav, waqk, xa1, xa2, xa3).  V for chunk n needs only
    # xa chunk n + v weights (shortest critical prefix starts PE earliest).
    # V is in [token, feature] layout: V_sb[:, r, :] = rows 128r..128r+127.
    V_sb = singles.tile([128, S // 128, HL * D], F32R)
    qkT_sb = singles.tile([128, 2 * HL, S], F32R)
    for n in range(G):
        for r in range(4 * n, 4 * n + 4):
            ps = psum.tile([128, R], F32, tag="st")
            for c in range(CK):
                nc.tensor.matmul(
                    ps[:, :HL * D],
                    lhsT=xa_sb[:, n, c, (r % 4) * 128:(r % 4 + 1) * 128],
                    rhs=wav_sb[:, c, :],
                    start=(c == 0),
                    stop=(c == CK - 1),
                )
            nc.vector.tensor_copy(V_sb[:, r, :], ps[:, :HL * D])
        for f in range(2 * HL):
            ps = psum.tile([128, R], F32, tag="st")
            for c in range(CK):
                nc.tensor.matmul(
                    ps,
                    lhsT=waqk_sb[:, c, f * 128:(f + 1) * 128],
                    rhs=xa_sb[:, n, c, :],
                    start=(c == 0),
                    stop=(c == CK - 1),
                )
            if f < HL:
                nc.scalar.add(qkT_sb[:, f, n * R:(n + 1) * R], ps, bq_sb[:, f:f + 1])
            else:
                nc.vector.tensor_copy(qkT_sb[:, f, n * R:(n + 1) * R], ps)

    # ---- attention + output projection, software-pipelined ----
    # Emission order == PE execution order (in-order engine).  Defer each
    # batch's PV/rowsum matmuls (which wait on its exp) by TWO score batches,
    # across head/group boundaries, so exp latency and the
    # recip/norm/proj chain never stall PE.
    pending = []
    proj_queue = []

    def push(fn):
        pending.append(fn)
        while len(pending) > 2:
            pending.pop(0)()

    def pop_proj(k):
        for _ in range(min(k, len(proj_queue))):
            proj_queue.pop(0)()

    def drain():
        while pending:
            pending.pop(0)()
        while proj_queue:
            proj_queue.pop(0)()

    for t in range(G):
        rows = slice(t * R, (t + 1) * R)
        nk = 4 * (t + 1)
        ao = aopool.tile([128, HL, R], F32R, tag="ao")
        for h in range(HL):
            pv = psum.tile([128, R], F32, tag="pv")
            rs = psum.tile([128, R], F32, tag="rs")
            for jb in range(nk // 2):
                if jb == 1:
                    pop_proj(2)  # head-start filler hides exp/norm latency
                st = psum.tile([128, 2, R], F32, tag="st")
                for u in range(2):
                    j = 2 * jb + u
                    nc.tensor.matmul(
                        st[:, u, :],
                        lhsT=qkT_sb[:, HL + h, j * 128:(j + 1) * 128],
                        rhs=qkT_sb[:, h, rows],
                        start=True,
                        stop=True,
                    )
                    if j >= nk - 4:
                        nc.vector.tensor_tensor(
                            st[:, u, :], st[:, u, :],
                            mask_ap(j - (nk - 4)), mybir.AluOpType.add,
                        )
                ex = expool.tile([128, 2, R], F32R, tag="ex")
                nc.scalar.activation(
                    ex, st, mybir.ActivationFunctionType.Exp, scale=INV_SQRT_D,
                )

                def consume(jb=jb, h=h, t=t, pv=pv, rs=rs, ex=ex, ao=ao, nk=nk):
                    for u in range(2):
                        j = 2 * jb + u
                        nc.tensor.matmul(
                            pv,
                            lhsT=V_sb[:, j, h * D:(h + 1) * D],
                            rhs=ex[:, u, :],
                            start=(j == 0),
                            stop=(j == nk - 1),
                        )
                        nc.tensor.matmul(
                            rs,
                            lhsT=ones_sb,
                            rhs=ex[:, u, :],
                            start=(j == 0),
                            stop=(j == nk - 1),
                        )
                    if jb == nk // 2 - 1:
                        rsr = rspool.tile([128, R], F32, tag="rsr")
                        nc.vector.reciprocal(rsr, rs)
                        nc.vector.tensor_tensor(
                            ao[:, h, :], pv, rsr, mybir.AluOpType.mult)
                        if h == HL - 1:
                            proj_queue.extend(
                                _proj_obs(nc, psum, otpool, wp_sb, ao, outT, t))

                push(consume)
    drain()


def _proj_obs(nc, psum, otpool, wp_sb, ao, outT, t):
    rows = slice(t * R, (t + 1) * R)

    def one(ob):
        ps = psum.tile([128, R], F32, tag="pv")
        for fc in range(HL):
            nc.tensor.matmul(
                ps,
                lhsT=wp_sb[:, fc, ob * 128:(ob + 1) * 128],
                rhs=ao[:, fc, :],
                start=(fc == 0),
                stop=(fc == HL - 1),
            )
        ot = otpool.tile([128, R], F32, tag="ot")
        if ob % 2 == 0:
            nc.scalar.copy(ot, ps)
        else:
            nc.vector.tensor_copy(ot, ps)
        nc.sync.dma_start(outT[ob * 128:(ob + 1) * 128, rows], ot)

    return [lambda ob=ob: one(ob) for ob in range(C // 128)]


_CACHED = None


def _build():
    global _CACHED
    if _CACHED is not None:
        return _CACHED
    nc = bacc.Bacc(
        "TRN2",
        target_bir_lowering=False,
        debug=False,
        enable_asserts=False,
        num_devices=N_CORES,
    )
    xa = nc.dram_tensor("xa", [128, G * CK * R], F32R, kind="ExternalInput").ap()
    wav = nc.dram_tensor("wav", [128, CK * HL * D], F32R, kind="ExternalInput").ap()
    waqk = nc.dram_tensor("waqk", [128, CK * 2 * HL * D], F32R, kind="ExternalInput").ap()
    aux = nc.dram_tensor("aux", [128, AUXW], F32, kind="ExternalInput").ap()
    ones = nc.dram_tensor("ones", [128, 128], F32R, kind="ExternalInput").ap()
    wp = nc.dram_tensor("wp", [128, HL * C], F32R, kind="ExternalInput").ap()
    outT = nc.dram_tensor("outT", [C, S], F32, kind="ExternalOutput").ap()
    with tile.TileContext(nc) as tc, ExitStack() as ctx:
        _emit(ctx, tc, xa, wav, waqk, aux, ones, wp, outT)
    nc.compile()
    _CACHED = nc
    return nc


def _pmajor(a2d):
    """[n*128, w] -> [128, n*w]: partition-major shuffle for one-DMA loads."""
    n = a2d.shape[0] // 128
    w = a2d.shape[1]
    return np.ascontiguousarray(
        a2d.reshape(n, 128, w).transpose(1, 0, 2).reshape(128, n * w))


def _masks():
    i = np.arange(R)[None, :]
    j = np.arange(128)[:, None]
    cols = [np.where(i >= j + 128 * k, 0.0, MASK_VAL).astype(np.float32)
            for k in range(4)]
    return np.concatenate(cols, axis=1)  # [128, 4*R]


def shard_inputs(x, w_attn, b_attn, w_proj):
    """Per-core input dicts for cores 0..7 (core = 2*batch + head_group)."""
    masks = _masks()
    ones = np.ones((128, 128), np.float32)
    in_maps = []
    for c in range(N_CORES):
        b, g = divmod(c, 2)
        lo, hi = g * HL * D, (g + 1) * HL * D
        wav = w_attn[:, 2 * C + lo:2 * C + hi]
        waqk = np.concatenate(
            [w_attn[:, lo:hi], w_attn[:, C + lo:C + hi]], axis=1)
        xT = np.ascontiguousarray(x[b].T)  # [768, 2048]
        xa = np.concatenate(
            [_pmajor(xT[:, n * R:(n + 1) * R]) for n in range(G)], axis=1)
        bq = np.ascontiguousarray(b_attn[lo:hi]).reshape(HL, 128).T  # [128, HL]
        in_maps.append({
            "xa": xa,
            "wav": _pmajor(wav),
            "waqk": _pmajor(waqk),
            "aux": np.concatenate([bq, masks], axis=1),
            "ones": ones,
            "wp": _pmajor(w_proj[lo:hi, :]),
        })
    return in_maps


def combine_outputs(parts, b_attn, w_proj, b_proj):
    """parts[c] = outT partial [768, 2048] from core c."""
    bias = b_attn[2 * C:].astype(np.float64) @ w_proj.astype(np.float64) + b_proj
    out = np.empty((B, S, C), np.float32)
    for b in range(B):
        acc = parts[2 * b].astype(np.float32) + parts[2 * b + 1]
        out[b] = acc.T + bias.astype(np.float32)[None, :]
    return out


def kernel(x, w_attn, b_attn, w_proj, b_proj, **run_kwargs):
    x = np.asarray(x, np.float32)
    w_attn = np.asarray(w_attn, np.float32)
    b_attn = np.asarray(b_attn, np.float32)
    w_proj = np.asarray(w_proj, np.float32)
    b_proj = np.asarray(b_proj, np.float32)

    nc = _build()
    in_maps = shard_inputs(x, w_attn, b_attn, w_proj)
    res = bass_utils.run_bass_kernel_spmd(
        nc, in_maps, core_ids=list(range(N_CORES)), **run_kwargs
    )
    parts = [r["outT"] for r in res.results]
    out = combine_outputs(parts, b_attn, w_proj, b_proj)
    kernel.last_results = res
    return out



# revision 19
# speedup vs baseline: 1.4140x; 1.4140x over previous
"""Causal multi-head attention on 8 trn2 NeuronCores.

Problem (hardcoded): x[4, 2048, 768], w_attn[768, 2304], b_attn[2304],
w_proj[768, 768], b_proj[768]; H=6 heads, D=128 head dim; fp32 in/out.

Sharding: core c = 2*b + g handles batch b and head-group g (heads
3g..3g+2).  Each core computes Q/K/V projections for its 3 heads over the
full sequence, full causal attention for those heads, and a PARTIAL output
projection (w_proj rows of its heads).  The host sums the two partials per
batch and adds the bias terms.  No cross-core communication.

Bias algebra (host/device split):
  - b_q is added on device (affects scores).
  - b_k is dropped entirely: it shifts every score in a softmax row by the
    same constant, which cancels.
  - b_v is dropped on device: softmax rows sum to 1, so attn @ (v + b_v)
    = attn @ v + b_v; the constant (b_v @ w_proj + b_proj) is added on host.

Precision plan (HW rel err ~3e-3 vs the 2e-2 gate):
  - QKV projection runs as fp8e4m3 DoubleRow matmuls (2 contraction chunks
    per pass, 0.5 cycles/row = 4x fp32r throughput) using a hi+lo residual
    split of BOTH x and w_attn: x ~= xh + xl, w ~= wh + wl (each fp8), and
    q = xh@wh + xh@wl + xl@wh (the lo*lo term is ~1e-5 and dropped).  Nine
    DR matmuls replace six bf16 ones: 25% fewer PE cycles, same DMA bytes
    as bf16, all splits prepared on the host.  w_attn is pre-scaled by 32
    so its residual stays out of the fp8 subnormal floor; the 32x on q/k
    is folded into the exp scale and the 32x on v into w_proj (pre-divided
    by 32 on the host).
  - Everything else is bf16 operands with fp32 PSUM accumulation: scores,
    PV, output projection (1 cycle/row, same as fp32r, but half the SBUF
    and DMA).  fp8 anywhere else fails the error gate (outlier softmax
    rows: max|out|/rms ~ 32).

Softmax denominators: instead of 40 ones-matmuls per head (as many PE
cycles as PV itself), the exp'd score chunks are accumulated on the DVE
into a per-(group,head) fp16 tile (2-byte dtype = 2x DVE mode), and ONE
ones-matmul per (group,head) does the final 128-partition reduction,
replicating the denominators across partitions for the divide.

Causal masking: scores/exp are computed on full 512-row rectangles
(group t attends kv [0, 512(t+1))); the 4 diagonal kv chunks are fixed by
multiplying ex (bf16, SBUF) with 0/1 bf16 masks trimmed to the first
128(k+1) columns — 2-byte all-SBUF operands run in the DVE 2x mode,
~3x cheaper than the old additive -1e9 mask on fp32 PSUM.

Engine budget per core (cost model): PE ~104us (QKV 35 + scores 26 + PV 26
+ rowsum 3 + proj 15), Act ~75us (exp), DVE ~70us, Pool ~40us (copies).

Scheduling: inputs split into several DMAs ordered by first use; the
attention inner loop keeps a two-batch software-pipeline skew (PE is
in-order) so exp latency and the recip/divide/proj chain never stall PE.
"""

import math
from contextlib import ExitStack

import numpy as np
import ml_dtypes

import concourse.bacc as bacc
import concourse.bass as bass
import concourse.mybir as mybir
import concourse.tile as tile
from concourse import bass_utils

B, S, C = 4, 2048, 768
H, D = 6, 128
HL = 3          # heads per core
CK = C // 128   # 6 contraction chunks
CP = CK // 2    # 3 chunk pairs for DoubleRow
R = 512         # query rows per group
G = S // R      # 4 groups
N_CORES = 8
F32 = mybir.dt.float32
BF16 = mybir.dt.bfloat16
FP16 = mybir.dt.float16
FP8 = mybir.dt.float8e4
NP_FP8 = ml_dtypes.float8_e4m3
NP_BF16 = ml_dtypes.bfloat16
WS = 32.0                      # w_attn pre-scale (fp8 subnormal avoidance)
INV_SQRT_D = 1.0 / math.sqrt(D)
EXP_SCALE = INV_SQRT_D / (WS * WS)
MASKW = 128                    # one shared [128,128] causal triangle mask
DR = mybir.MatmulPerfMode.DoubleRow


def _emit(ctx: ExitStack, tc: tile.TileContext, xah, xal, wavh, wavl,
          wqkh, wqkl, bq, mask01, ones, wp, outT):
    nc = tc.nc

    singles = ctx.enter_context(tc.tile_pool(name="singles", bufs=1))
    expool = ctx.enter_context(tc.tile_pool(name="expool", bufs=2))
    aopool = ctx.enter_context(tc.tile_pool(name="aopool", bufs=2))
    otpool = ctx.enter_context(tc.tile_pool(name="otpool", bufs=2))
    rspool = ctx.enter_context(tc.tile_pool(name="rspool", bufs=2))
    accpool = ctx.enter_context(tc.tile_pool(name="accpool", bufs=2))
    psum = ctx.enter_context(tc.tile_pool(name="psum", bufs=2, space="PSUM"))

    # ---- resident loads, split + ordered by first use ----
    xah_sb = singles.tile([128, G, CK, R], FP8)
    xal_sb = singles.tile([128, G, CK, R], FP8)
    wavh_sb = singles.tile([128, CK, HL * D], FP8)
    wavl_sb = singles.tile([128, CK, HL * D], FP8)
    wqkh_sb = singles.tile([128, CK, 2 * HL * D], FP8)
    wqkl_sb = singles.tile([128, CK, 2 * HL * D], FP8)
    bq_sb = singles.tile([128, HL], F32)
    mask_sb = singles.tile([128, MASKW], BF16)
    ones_sb = singles.tile([128, 128], FP16)
    wp_sb = singles.tile([128, HL, C], BF16)

    def load_group(xa_sb, xa, n):
        nc.sync.dma_start(
            xa_sb[:, n],
            xa[:, n * CK * R:(n + 1) * CK * R].rearrange("p (c s) -> p c s", c=CK))

    load_group(xah_sb, xah, 0)
    load_group(xal_sb, xal, 0)
    nc.sync.dma_start(wavh_sb, wavh.rearrange("p (c n) -> p c n", c=CK))
    nc.sync.dma_start(wavl_sb, wavl.rearrange("p (c n) -> p c n", c=CK))
    nc.sync.dma_start(wqkh_sb, wqkh.rearrange("p (c n) -> p c n", c=CK))
    nc.sync.dma_start(wqkl_sb, wqkl.rearrange("p (c n) -> p c n", c=CK))
    nc.sync.dma_start(bq_sb, bq)
    nc.sync.dma_start(mask_sb, mask01)
    nc.sync.dma_start(ones_sb, ones)
    for n in range(1, G):
        load_group(xah_sb, xah, n)
        load_group(xal_sb, xal, n)
    nc.sync.dma_start(wp_sb, wp.rearrange("p (f n) -> p f n", f=HL))

    # ---- QKV projections: 9 fp8 DoubleRow matmuls per output tile
    # (3 chunk pairs x 3 hi/lo cross terms), interleaved per x token-chunk
    # so PE work tracks DMA arrival.
    V_sb = singles.tile([128, S // 128, HL * D], BF16)
    qkT_sb = singles.tile([128, 2 * HL, S], BF16)
    for n in range(G):
        for r in range(4 * n, 4 * n + 4):
            ps = psum.tile([128, R], F32, tag="st")
            idx = 0
            for xs, ws in ((xah_sb, wavh_sb), (xah_sb, wavl_sb),
                           (xal_sb, wavh_sb)):
                for cp in range(CP):
                    nc.tensor.matmul(
                        ps[:, :HL * D],
                        lhsT=xs[:, n, 2 * cp:2 * cp + 2,
                                (r % 4) * 128:(r % 4 + 1) * 128],
                        rhs=ws[:, 2 * cp:2 * cp + 2, :],
                        start=(idx == 0),
                        stop=(idx == 3 * CP - 1),
                        perf_mode=DR,
                    )
                    idx += 1
            nc.scalar.copy(V_sb[:, r, :], ps[:, :HL * D])
        for f in range(2 * HL):
            ps = psum.tile([128, R], F32, tag="st")
            idx = 0
            for ws, xs in ((wqkh_sb, xah_sb), (wqkl_sb, xah_sb),
                           (wqkh_sb, xal_sb)):
                for cp in range(CP):
                    nc.tensor.matmul(
                        ps,
                        lhsT=ws[:, 2 * cp:2 * cp + 2, f * 128:(f + 1) * 128],
                        rhs=xs[:, n, 2 * cp:2 * cp + 2, :],
                        start=(idx == 0),
                        stop=(idx == 3 * CP - 1),
                        perf_mode=DR,
                    )
                    idx += 1
            if f < HL:
                nc.scalar.add(qkT_sb[:, f, n * R:(n + 1) * R], ps, bq_sb[:, f:f + 1])
            else:
                nc.scalar.copy(qkT_sb[:, f, n * R:(n + 1) * R], ps)

    # ---- attention + output projection, software-pipelined ----
    pending = []
    proj_queue = []

    def push(fn):
        pending.append(fn)
        while len(pending) > 2:
            pending.pop(0)()

    def pop_proj(k):
        for _ in range(min(k, len(proj_queue))):
            proj_queue.pop(0)()

    def drain():
        while pending:
            pending.pop(0)()
        while proj_queue:
            proj_queue.pop(0)()

    # Diagonal kv chunk k (k = j-(nk-4)) only attends query columns
    # [128k, 512): scores/exp/PV/acc all run on that suffix, and the causal
    # triangle inside its first 128 columns is fixed by ONE shared [128,128]
    # 0/1 mask.  Chunk j=0 is always full-width, so the PV psum start=True
    # initializes the whole [128, R] region.
    for t in (0, 1, 2, 3):
        rows = t * R
        nk = 4 * (t + 1)

        def off(j, nk=nk):
            return 128 * (j - (nk - 4)) if j >= nk - 4 else 0

        ao = aopool.tile([128, HL, R], BF16, tag="ao")
        for h in range(HL):
            pv = psum.tile([128, R], F32, tag="pv")
            acc = accpool.tile([128, R], FP16, tag="acc")
            for jb in range(nk // 2):
                if jb == 1:
                    pop_proj(2)  # head-start filler hides exp/norm latency
                o0, o1 = off(2 * jb), off(2 * jb + 1)
                st = psum.tile([128, 2, R], F32, tag="st")
                for u, o in ((0, o0), (1, o1)):
                    j = 2 * jb + u
                    nc.tensor.matmul(
                        st[:, u, o:],
                        lhsT=qkT_sb[:, HL + h, j * 128:(j + 1) * 128],
                        rhs=qkT_sb[:, h, rows + o:rows + R],
                        start=True,
                        stop=True,
                    )
                ex = expool.tile([128, 2, R], BF16, tag="ex")
                # one exp over the common suffix, plus chunk0's head columns
                nc.scalar.activation(
                    ex[:, :, o1:], st[:, :, o1:],
                    mybir.ActivationFunctionType.Exp, scale=EXP_SCALE,
                )
                if o1 > o0:
                    nc.scalar.activation(
                        ex[:, 0, o0:o1], st[:, 0, o0:o1],
                        mybir.ActivationFunctionType.Exp, scale=EXP_SCALE,
                    )
                # causal fix-up: 0/1 triangle mult on each diagonal chunk's
                # first 128 valid columns (bf16 all-SBUF = DVE 2x mode)
                for u, o in ((0, o0), (1, o1)):
                    j = 2 * jb + u
                    if j >= nk - 4:
                        nc.vector.tensor_tensor(
                            ex[:, u, o:o + 128], ex[:, u, o:o + 128],
                            mask_sb, mybir.AluOpType.mult,
                        )
                # fp16 denominator accumulation on DVE (2x mode): tmp is
                # independent per jb; only the acc += tmp adds are chained
                if jb == 0:
                    if o1 == 0:
                        nc.vector.tensor_tensor(
                            acc, ex[:, 0, :], ex[:, 1, :], mybir.AluOpType.add)
                    else:  # t == 0: ragged first pair
                        nc.vector.tensor_copy(acc, ex[:, 0, :])
                        nc.vector.tensor_tensor(
                            acc[:, o1:], acc[:, o1:], ex[:, 1, o1:],
                            mybir.AluOpType.add)
                elif o0 == o1 == 0:
                    tmp = accpool.tile([128, R], FP16, tag="tmp")
                    nc.vector.tensor_tensor(
                        tmp, ex[:, 0, :], ex[:, 1, :], mybir.AluOpType.add)
                    nc.vector.tensor_tensor(acc, acc, tmp, mybir.AluOpType.add)
                else:  # diagonal pair: two direct suffix adds
                    nc.vector.tensor_tensor(
                        acc[:, o0:], acc[:, o0:], ex[:, 0, o0:],
                        mybir.AluOpType.add)
                    nc.vector.tensor_tensor(
                        acc[:, o1:], acc[:, o1:], ex[:, 1, o1:],
                        mybir.AluOpType.add)

                def consume(jb=jb, h=h, pv=pv, ex=ex, nk=nk, o0=o0, o1=o1):
                    for u, o in ((0, o0), (1, o1)):
                        j = 2 * jb + u
                        nc.tensor.matmul(
                            pv[:, o:],
                            lhsT=V_sb[:, j, h * D:(h + 1) * D],
                            rhs=ex[:, u, o:],
                            start=(j == 0),
                            stop=(j == nk - 1),
                        )

                push(consume)

            def finalize(h=h, t=t, pv=pv, acc=acc, ao=ao):
                rs = psum.tile([128, R], F32, tag="rs")
                nc.tensor.matmul(rs, lhsT=ones_sb, rhs=acc, start=True, stop=True)
                rsr = rspool.tile([128, R], F32, tag="rsr")
                nc.vector.reciprocal(rsr, rs)
                nc.vector.tensor_tensor(
                    ao[:, h, :], pv, rsr, mybir.AluOpType.mult)
                if h == HL - 1:
                    proj_queue.extend(
                        _proj_obs(nc, psum, otpool, wp_sb, ao, outT, t))

            push(finalize)
    drain()


def _proj_obs(nc, psum, otpool, wp_sb, ao, outT, t):
    rows = slice(t * R, (t + 1) * R)

    def one(ob):
        ps = psum.tile([128, R], mybir.dt.float32, tag="pv")
        for fc in range(HL):
            nc.tensor.matmul(
                ps,
                lhsT=wp_sb[:, fc, ob * 128:(ob + 1) * 128],
                rhs=ao[:, fc, :],
                start=(fc == 0),
                stop=(fc == HL - 1),
            )
        ot = otpool.tile([128, R], BF16, tag="ot")
        nc.vector.tensor_copy(ot, ps)
        nc.sync.dma_start(outT[ob * 128:(ob + 1) * 128, rows], ot)

    return [lambda ob=ob: one(ob) for ob in range(C // 128)]


_CACHED = None


def _build():
    global _CACHED
    if _CACHED is not None:
        return _CACHED
    nc = bacc.Bacc(
        "TRN2",
        target_bir_lowering=False,
        debug=False,
        enable_asserts=False,
        num_devices=N_CORES,
    )
    xah = nc.dram_tensor("xah", [128, G * CK * R], FP8, kind="ExternalInput").ap()
    xal = nc.dram_tensor("xal", [128, G * CK * R], FP8, kind="ExternalInput").ap()
    wavh = nc.dram_tensor("wavh", [128, CK * HL * D], FP8, kind="ExternalInput").ap()
    wavl = nc.dram_tensor("wavl", [128, CK * HL * D], FP8, kind="ExternalInput").ap()
    wqkh = nc.dram_tensor("wqkh", [128, CK * 2 * HL * D], FP8, kind="ExternalInput").ap()
    wqkl = nc.dram_tensor("wqkl", [128, CK * 2 * HL * D], FP8, kind="ExternalInput").ap()
    bq = nc.dram_tensor("bq", [128, HL], F32, kind="ExternalInput").ap()
    mask01 = nc.dram_tensor("mask01", [128, MASKW], BF16, kind="ExternalInput").ap()
    ones = nc.dram_tensor("ones", [128, 128], FP16, kind="ExternalInput").ap()
    wp = nc.dram_tensor("wp", [128, HL * C], BF16, kind="ExternalInput").ap()
    outT = nc.dram_tensor("outT", [C, S], BF16, kind="ExternalOutput").ap()
    with tile.TileContext(nc) as tc, ExitStack() as ctx:
        _emit(ctx, tc, xah, xal, wavh, wavl, wqkh, wqkl, bq, mask01, ones, wp, outT)
    nc.compile()
    _CACHED = nc
    return nc


def _pmajor(a2d):
    """[n*128, w] -> [128, n*w]: partition-major shuffle for one-DMA loads."""
    n = a2d.shape[0] // 128
    w = a2d.shape[1]
    return np.ascontiguousarray(
        a2d.reshape(n, 128, w).transpose(1, 0, 2).reshape(128, n * w))


def _hilo(a):
    """fp32 array -> (hi, lo) fp8e4m3 pair with hi + lo ~= a."""
    hi = a.astype(NP_FP8)
    lo = (a - hi.astype(np.float32)).astype(NP_FP8)
    return hi, lo


def _masks01():
    """Shared 0/1 bf16 causal triangle: mask[p, q] = 1 iff q >= p."""
    p = np.arange(128)[:, None]
    q = np.arange(128)[None, :]
    return (q >= p).astype(NP_BF16)  # [128, 128]


def shard_inputs(x, w_attn, b_attn, w_proj):
    """Per-core input dicts for cores 0..7 (core = 2*batch + head_group)."""
    masks = _masks01()
    ones = np.ones((128, 128), np.float16)
    xs = []
    for b in range(B):
        xT = np.ascontiguousarray(x[b].T)  # [768, 2048]
        xh, xl = _hilo(xT)
        xs.append(tuple(
            np.concatenate(
                [_pmajor(xq[:, n * R:(n + 1) * R]) for n in range(G)], axis=1)
            for xq in (xh, xl)))
    in_maps = []
    for c in range(N_CORES):
        b, g = divmod(c, 2)
        lo, hi = g * HL * D, (g + 1) * HL * D
        wav = w_attn[:, 2 * C + lo:2 * C + hi] * WS
        wqk = np.concatenate(
            [w_attn[:, lo:hi], w_attn[:, C + lo:C + hi]], axis=1) * WS
        wavh, wavl = _hilo(wav)
        wqkh, wqkl = _hilo(wqk)
        bqv = (WS * b_attn[lo:hi]).astype(np.float32).reshape(HL, 128).T
        in_maps.append({
            "xah": xs[b][0],
            "xal": xs[b][1],
            "wavh": _pmajor(wavh),
            "wavl": _pmajor(wavl),
            "wqkh": _pmajor(wqkh),
            "wqkl": _pmajor(wqkl),
            "bq": np.ascontiguousarray(bqv),
            "mask01": masks,
            "ones": ones,
            "wp": _pmajor((w_proj[lo:hi, :] / WS).astype(NP_BF16)),
        })
    return in_maps


def combine_outputs(parts, b_attn, w_proj, b_proj):
    """parts[c] = outT partial [768, 2048] (bf16) from core c."""
    bias = b_attn[2 * C:].astype(np.float64) @ w_proj.astype(np.float64) + b_proj
    out = np.empty((B, S, C), np.float32)
    for b in range(B):
        acc = parts[2 * b].astype(np.float32) + parts[2 * b + 1].astype(np.float32)
        out[b] = acc.T + bias.astype(np.float32)[None, :]
    return out


def kernel(x, w_attn, b_attn, w_proj, b_proj, **run_kwargs):
    x = np.asarray(x, np.float32)
    w_attn = np.asarray(w_attn, np.float32)
    b_attn = np.asarray(b_attn, np.float32)
    w_proj = np.asarray(w_proj, np.float32)
    b_proj = np.asarray(b_proj, np.float32)

    nc = _build()
    in_maps = shard_inputs(x, w_attn, b_attn, w_proj)
    res = bass_utils.run_bass_kernel_spmd(
        nc, in_maps, core_ids=list(range(N_CORES)), **run_kwargs
    )
    parts = [r["outT"] for r in res.results]
    out = combine_outputs(parts, b_attn, w_proj, b_proj)
    kernel.last_results = res
    return out


# revision 45
# speedup vs baseline: 1.4960x; 1.0580x over previous
"""Causal multi-head attention on 8 trn2 NeuronCores.

Problem (hardcoded): x[4, 2048, 768], w_attn[768, 2304], b_attn[2304],
w_proj[768, 768], b_proj[768]; H=6 heads, D=128 head dim; fp32 in/out.

Sharding: core c = 2*b + g handles batch b and head-group g (heads
3g..3g+2).  Each core computes Q/K/V projections for its 3 heads over the
full sequence, full causal attention for those heads, and a PARTIAL output
projection (w_proj rows of its heads).  The host sums the two partials per
batch and adds the bias terms.  No cross-core communication.

Bias algebra (host/device split):
  - b_q is added on device (affects scores).
  - b_k is dropped entirely: it shifts every score in a softmax row by the
    same constant, which cancels.
  - b_v is dropped on device: softmax rows sum to 1, so attn @ (v + b_v)
    = attn @ v + b_v; the constant (b_v @ w_proj + b_proj) is added on host.

Precision plan (HW rel err ~3e-3 vs the 2e-2 gate):
  - QKV projection runs as fp8e4m3 DoubleRow matmuls (2 contraction chunks
    per pass, 0.5 cycles/row = 4x fp32r throughput) using a hi+lo residual
    split of BOTH x and w_attn: x ~= xh + xl, w ~= wh + wl (each fp8), and
    q = xh@wh + xh@wl + xl@wh (the lo*lo term is ~1e-5 and dropped).  Nine
    DR matmuls replace six bf16 ones: 25% fewer PE cycles, same DMA bytes
    as bf16, all splits prepared on the host.  w_attn is pre-scaled by 32
    so its residual stays out of the fp8 subnormal floor; the 32x on q/k
    is folded into the exp scale and the 32x on v into w_proj (pre-divided
    by 32 on the host).
  - Everything else is bf16 operands with fp32 PSUM accumulation: scores,
    PV, output projection (1 cycle/row, same as fp32r, but half the SBUF
    and DMA).  fp8 anywhere else fails the error gate (outlier softmax
    rows: max|out|/rms ~ 32).

Softmax denominators: instead of 40 ones-matmuls per head (as many PE
cycles as PV itself), the exp'd score chunks are accumulated on the DVE
into a per-(group,head) fp16 tile (2-byte dtype = 2x DVE mode), and ONE
ones-matmul per (group,head) does the final 128-partition reduction,
replicating the denominators across partitions for the divide.

Causal masking: scores/exp are computed on full 512-row rectangles
(group t attends kv [0, 512(t+1))); the 4 diagonal kv chunks are fixed by
multiplying ex (bf16, SBUF) with 0/1 bf16 masks trimmed to the first
128(k+1) columns — 2-byte all-SBUF operands run in the DVE 2x mode,
~3x cheaper than the old additive -1e9 mask on fp32 PSUM.

Engine budget per core (cost model): PE ~104us (QKV 35 + scores 26 + PV 26
+ rowsum 3 + proj 15), Act ~75us (exp), DVE ~70us, Pool ~40us (copies).

Scheduling: inputs split into several DMAs ordered by first use; the
attention inner loop keeps a two-batch software-pipeline skew (PE is
in-order) so exp latency and the recip/divide/proj chain never stall PE.
"""

import math
from contextlib import ExitStack

import numpy as np
import ml_dtypes

import concourse.bacc as bacc
import concourse.bass as bass
import concourse.mybir as mybir
import concourse.tile as tile
from concourse import bass_utils

B, S, C = 4, 2048, 768
H, D = 6, 128
HL = 3          # heads per core
CK = C // 128   # 6 contraction chunks
CP = CK // 2    # 3 chunk pairs for DoubleRow
R = 512         # query rows per group
G = S // R      # 4 groups
N_CORES = 8
F32 = mybir.dt.float32
BF16 = mybir.dt.bfloat16
FP16 = mybir.dt.float16
FP8 = mybir.dt.float8e4
NP_FP8 = ml_dtypes.float8_e4m3
NP_BF16 = ml_dtypes.bfloat16
WS = 32.0                      # w_attn pre-scale (fp8 subnormal avoidance)
INV_SQRT_D = 1.0 / math.sqrt(D)
EXP_SCALE = INV_SQRT_D / (WS * WS)
MASKW = 128                    # one shared [128,128] causal triangle mask
DR = mybir.MatmulPerfMode.DoubleRow


def _emit(ctx: ExitStack, tc: tile.TileContext, xa, wav, wqk,
          bq, mask01, ones, wp, outT):
    nc = tc.nc

    singles = ctx.enter_context(tc.tile_pool(name="singles", bufs=1))
    expool = ctx.enter_context(tc.tile_pool(name="expool", bufs=8))
    aopool = ctx.enter_context(tc.tile_pool(name="aopool", bufs=2))
    otpool = ctx.enter_context(tc.tile_pool(name="otpool", bufs=4))
    rspool = ctx.enter_context(tc.tile_pool(name="rspool", bufs=3))
    accpool = ctx.enter_context(tc.tile_pool(name="accpool", bufs=3))
    psum = ctx.enter_context(tc.tile_pool(name="psum", bufs=2, space="PSUM"))

    # ---- resident loads, split + ordered by first use ----
    xa_sb = singles.tile([128, G, 2, CK, R], FP8)
    wav_sb = singles.tile([128, 2, CK, HL * D], FP8)
    wqk_sb = singles.tile([128, 2, CK, 2 * HL * D], FP8)
    bq_sb = singles.tile([128, HL], F32)
    mask_sb = singles.tile([128, MASKW], BF16)
    ones_sb = singles.tile([128, 128], FP16)
    wp_sb = singles.tile([128, HL, C], BF16)

    def load_half(n, hi):
        w, half = 2 * CK * R, CK * R
        nc.sync.dma_start(
            xa_sb[:, n, hi],
            xa[:, n * w + hi * half:n * w + (hi + 1) * half]
            .rearrange("p (c s) -> p c s", c=CK))

    def load_whalf(w_sb, w_dram, hi, width):
        nc.sync.dma_start(
            w_sb[:, hi],
            w_dram[:, hi * CK * width:(hi + 1) * CK * width]
            .rearrange("p (c n) -> p c n", c=CK))

    # ordered by first use: V tiles run hi*hi first (xa0h+wavh), then
    # hi*lo (wavl), then lo*hi (xa0l); QK tiles follow (wqk hi, lo)
    load_half(0, 0)
    load_whalf(wav_sb, wav, 0, HL * D)
    load_whalf(wav_sb, wav, 1, HL * D)
    load_half(0, 1)
    load_whalf(wqk_sb, wqk, 0, 2 * HL * D)
    load_whalf(wqk_sb, wqk, 1, 2 * HL * D)
    nc.sync.dma_start(bq_sb, bq)
    nc.sync.dma_start(mask_sb, mask01)
    nc.sync.dma_start(ones_sb, ones)
    for n in range(1, G):
        load_half(n, 0)
        load_half(n, 1)
    nc.sync.dma_start(wp_sb, wp.rearrange("p (f n) -> p f n", f=HL))

    # ---- QKV projections: 9 fp8 DoubleRow matmuls per output tile
    # (3 chunk pairs x 3 hi/lo cross terms), interleaved per x token-chunk
    # so PE work tracks DMA arrival.
    V_sb = singles.tile([128, S // 128, HL * D], BF16)
    qkT_sb = singles.tile([128, 2 * HL, S], BF16)
    for n in range(G):
        for r in range(4 * n, 4 * n + 4):
            ps = psum.tile([128, R], F32, tag="st")
            idx = 0
            for xi, wi in ((0, 0), (0, 1), (1, 0)):
                for cp in range(CP):
                    nc.tensor.matmul(
                        ps[:, :HL * D],
                        lhsT=xa_sb[:, n, xi, 2 * cp:2 * cp + 2,
                                   (r % 4) * 128:(r % 4 + 1) * 128],
                        rhs=wav_sb[:, wi, 2 * cp:2 * cp + 2, :],
                        start=(idx == 0),
                        stop=(idx == 3 * CP - 1),
                        perf_mode=DR,
                    )
                    idx += 1
            nc.scalar.copy(V_sb[:, r, :], ps[:, :HL * D])
        for f in range(2 * HL):
            ps = psum.tile([128, R], F32, tag="st")
            idx = 0
            for wi, xi in ((0, 0), (1, 0), (0, 1)):
                for cp in range(CP):
                    nc.tensor.matmul(
                        ps,
                        lhsT=wqk_sb[:, wi, 2 * cp:2 * cp + 2, f * 128:(f + 1) * 128],
                        rhs=xa_sb[:, n, xi, 2 * cp:2 * cp + 2, :],
                        start=(idx == 0),
                        stop=(idx == 3 * CP - 1),
                        perf_mode=DR,
                    )
                    idx += 1
            if f < HL:
                nc.scalar.add(qkT_sb[:, f, n * R:(n + 1) * R], ps, bq_sb[:, f:f + 1])
            else:
                nc.scalar.copy(qkT_sb[:, f, n * R:(n + 1) * R], ps)

    # ---- attention + output projection, software-pipelined ----
    pending = []
    proj_queue = []

    def push(fn):
        pending.append(fn)
        while len(pending) > 2:
            pending.pop(0)()

    def pop_proj(k):
        for _ in range(min(k, len(proj_queue))):
            proj_queue.pop(0)()

    def drain():
        while pending:
            pending.pop(0)()
        while proj_queue:
            proj_queue.pop(0)()

    # Diagonal kv chunk k (k = j-(nk-4)) only attends query columns
    # [128k, 512): scores/exp/PV/acc all run on that suffix, and the causal
    # triangle inside its first 128 columns is fixed by ONE shared [128,128]
    # 0/1 mask.  Chunk j=0 is always full-width, so the PV psum start=True
    # initializes the whole [128, R] region.
    for t in (0, 1, 2, 3):
        rows = t * R
        nk = 4 * (t + 1)

        def off(j, nk=nk):
            return 128 * (j - (nk - 4)) if j >= nk - 4 else 0

        ao = aopool.tile([128, HL, R], BF16, tag="ao")
        for h in range(HL):
            pv = psum.tile([128, R], F32, tag="pv")
            acc = accpool.tile([128, R], FP16, tag="acc")
            for jb in range(nk // 2):
                if jb >= nk // 2 - (3 if t == 3 else 2):
                    pop_proj(1)  # late fillers land where the Act lag peaks
                o0, o1 = off(2 * jb), off(2 * jb + 1)
                st = psum.tile([128, 2, R], F32, tag="st")
                for u, o in ((0, o0), (1, o1)):
                    j = 2 * jb + u
                    nc.tensor.matmul(
                        st[:, u, o:],
                        lhsT=qkT_sb[:, HL + h, j * 128:(j + 1) * 128],
                        rhs=qkT_sb[:, h, rows + o:rows + R],
                        start=True,
                        stop=True,
                    )
                ex = expool.tile([128, 2, R], BF16, tag="ex")
                # one exp over the common suffix, plus chunk0's head columns
                nc.scalar.activation(
                    ex[:, :, o1:], st[:, :, o1:],
                    mybir.ActivationFunctionType.Exp, scale=EXP_SCALE,
                )
                if o1 > o0:
                    nc.scalar.activation(
                        ex[:, 0, o0:o1], st[:, 0, o0:o1],
                        mybir.ActivationFunctionType.Exp, scale=EXP_SCALE,
                    )
                # causal fix-up: 0/1 triangle mult on each diagonal chunk's
                # first 128 valid columns (bf16 all-SBUF = DVE 2x mode)
                for u, o in ((0, o0), (1, o1)):
                    j = 2 * jb + u
                    if j >= nk - 4:
                        nc.vector.tensor_tensor(
                            ex[:, u, o:o + 128], ex[:, u, o:o + 128],
                            mask_sb, mybir.AluOpType.mult,
                        )
                # fp16 denominator accumulation on DVE (2x mode): tmp is
                # independent per jb; only the acc += tmp adds are chained
                if jb == 0:
                    if o1 == 0:
                        nc.vector.tensor_tensor(
                            acc, ex[:, 0, :], ex[:, 1, :], mybir.AluOpType.add)
                    else:  # t == 0: ragged first pair
                        nc.vector.tensor_copy(acc, ex[:, 0, :])
                        nc.vector.tensor_tensor(
                            acc[:, o1:], acc[:, o1:], ex[:, 1, o1:],
                            mybir.AluOpType.add)
                elif o0 == o1 == 0:
                    tmp = accpool.tile([128, R], FP16, tag="tmp")
                    nc.vector.tensor_tensor(
                        tmp, ex[:, 0, :], ex[:, 1, :], mybir.AluOpType.add)
                    nc.vector.tensor_tensor(acc, acc, tmp, mybir.AluOpType.add)
                else:  # diagonal pair: two direct suffix adds
                    nc.vector.tensor_tensor(
                        acc[:, o0:], acc[:, o0:], ex[:, 0, o0:],
                        mybir.AluOpType.add)
                    nc.vector.tensor_tensor(
                        acc[:, o1:], acc[:, o1:], ex[:, 1, o1:],
                        mybir.AluOpType.add)

                def consume(jb=jb, h=h, pv=pv, ex=ex, nk=nk, o0=o0, o1=o1):
                    for u, o in ((0, o0), (1, o1)):
                        j = 2 * jb + u
                        nc.tensor.matmul(
                            pv[:, o:],
                            lhsT=V_sb[:, j, h * D:(h + 1) * D],
                            rhs=ex[:, u, o:],
                            start=(j == 0),
                            stop=(j == nk - 1),
                        )

                push(consume)

            def finalize(h=h, t=t, pv=pv, acc=acc, ao=ao):
                rs = psum.tile([128, R], F32, tag="pv")
                nc.tensor.matmul(rs, lhsT=ones_sb, rhs=acc, start=True, stop=True)
                rsr = rspool.tile([128, R], F32, tag="rsr")
                nc.vector.reciprocal(rsr, rs)
                nc.vector.tensor_tensor(
                    ao[:, h, :], pv, rsr, mybir.AluOpType.mult)
                if h == HL - 1:
                    proj_queue.extend(
                        _proj_obs(nc, psum, otpool, wp_sb, ao, outT, t))

            push(finalize)
    drain()


def _proj_obs(nc, psum, otpool, wp_sb, ao, outT, t):
    rows = slice(t * R, (t + 1) * R)
    last = t == G - 1

    def one(ob):
        ps = psum.tile([128, R], mybir.dt.float32, tag="pv")
        for fc in range(HL):
            nc.tensor.matmul(
                ps,
                lhsT=wp_sb[:, fc, ob * 128:(ob + 1) * 128],
                rhs=ao[:, fc, :],
                start=(fc == 0),
                stop=(fc == HL - 1),
            )
        ot = otpool.tile([128, R], BF16, tag="ot")
        # last group's copies drain after all other work: split across
        # Act (idle by then) and DVE so the tail halves
        if last and ob % 2 == 0:
            nc.scalar.copy(ot, ps)
        else:
            nc.vector.tensor_copy(ot, ps)
        nc.sync.dma_start(outT[ob * 128:(ob + 1) * 128, rows], ot)

    return [lambda ob=ob: one(ob) for ob in range(C // 128)]


_CACHED = None


def _build():
    global _CACHED
    if _CACHED is not None:
        return _CACHED
    nc = bacc.Bacc(
        "TRN2",
        target_bir_lowering=False,
        debug=False,
        enable_asserts=False,
        num_devices=N_CORES,
    )
    xa = nc.dram_tensor("xa", [128, G * 2 * CK * R], FP8, kind="ExternalInput").ap()
    wav = nc.dram_tensor("wav", [128, 2 * CK * HL * D], FP8, kind="ExternalInput").ap()
    wqk = nc.dram_tensor("wqk", [128, 2 * CK * 2 * HL * D], FP8, kind="ExternalInput").ap()
    bq = nc.dram_tensor("bq", [128, HL], F32, kind="ExternalInput").ap()
    mask01 = nc.dram_tensor("mask01", [128, MASKW], BF16, kind="ExternalInput").ap()
    ones = nc.dram_tensor("ones", [128, 128], FP16, kind="ExternalInput").ap()
    wp = nc.dram_tensor("wp", [128, HL * C], BF16, kind="ExternalInput").ap()
    outT = nc.dram_tensor("outT", [C, S], BF16, kind="ExternalOutput").ap()
    with tile.TileContext(nc) as tc, ExitStack() as ctx:
        _emit(ctx, tc, xa, wav, wqk, bq, mask01, ones, wp, outT)
    nc.compile()
    _CACHED = nc
    return nc


def _pmajor(a2d):
    """[n*128, w] -> [128, n*w]: partition-major shuffle for one-DMA loads."""
    n = a2d.shape[0] // 128
    w = a2d.shape[1]
    return np.ascontiguousarray(
        a2d.reshape(n, 128, w).transpose(1, 0, 2).reshape(128, n * w))


def _hilo(a):
    """fp32 array -> (hi, lo) fp8e4m3 pair with hi + lo ~= a."""
    hi = a.astype(NP_FP8)
    lo = (a - hi.astype(np.float32)).astype(NP_FP8)
    return hi, lo


def _masks01():
    """Shared 0/1 bf16 causal triangle: mask[p, q] = 1 iff q >= p."""
    p = np.arange(128)[:, None]
    q = np.arange(128)[None, :]
    return (q >= p).astype(NP_BF16)  # [128, 128]


def shard_inputs(x, w_attn, b_attn, w_proj):
    """Per-core input dicts for cores 0..7 (core = 2*batch + head_group)."""
    masks = _masks01()
    ones = np.ones((128, 128), np.float16)
    xs = []
    for b in range(B):
        xT = np.ascontiguousarray(x[b].T)  # [768, 2048]
        xh, xl = _hilo(xT)
        # per group: [hi block | lo block], each chunk-major
        xs.append(np.concatenate(
            [_pmajor(xq[:, n * CK * 0 + n * R:(n + 1) * R])
             for n in range(G) for xq in (xh, xl)], axis=1))
    in_maps = []
    for c in range(N_CORES):
        b, g = divmod(c, 2)
        lo, hi = g * HL * D, (g + 1) * HL * D
        wav = w_attn[:, 2 * C + lo:2 * C + hi] * WS
        wqk = np.concatenate(
            [w_attn[:, lo:hi], w_attn[:, C + lo:C + hi]], axis=1) * WS
        wavh, wavl = _hilo(wav)
        wqkh, wqkl = _hilo(wqk)
        bqv = (WS * b_attn[lo:hi]).astype(np.float32).reshape(HL, 128).T
        in_maps.append({
            "xa": xs[b],
            "wav": np.concatenate([_pmajor(wavh), _pmajor(wavl)], axis=1),
            "wqk": np.concatenate([_pmajor(wqkh), _pmajor(wqkl)], axis=1),
            "bq": np.ascontiguousarray(bqv),
            "mask01": masks,
            "ones": ones,
            "wp": _pmajor((w_proj[lo:hi, :] / WS).astype(NP_BF16)),
        })
    return in_maps


def combine_outputs(parts, b_attn, w_proj, b_proj):
    """parts[c] = outT partial [768, 2048] (bf16) from core c."""
    bias = b_attn[2 * C:].astype(np.float64) @ w_proj.astype(np.float64) + b_proj
    out = np.empty((B, S, C), np.float32)
    for b in range(B):
        acc = parts[2 * b].astype(np.float32) + parts[2 * b + 1].astype(np.float32)
        out[b] = acc.T + bias.astype(np.float32)[None, :]
    return out


def kernel(x, w_attn, b_attn, w_proj, b_proj, **run_kwargs):
    x = np.asarray(x, np.float32)
    w_attn = np.asarray(w_attn, np.float32)
    b_attn = np.asarray(b_attn, np.float32)
    w_proj = np.asarray(w_proj, np.float32)
    b_proj = np.asarray(b_proj, np.float32)

    nc = _build()
    in_maps = shard_inputs(x, w_attn, b_attn, w_proj)
    res = bass_utils.run_bass_kernel_spmd(
        nc, in_maps, core_ids=list(range(N_CORES)), **run_kwargs
    )
    parts = [r["outT"] for r in res.results]
    out = combine_outputs(parts, b_attn, w_proj, b_proj)
    kernel.last_results = res
    return out


# revision 57
# speedup vs baseline: 1.5224x; 1.0176x over previous
"""Causal multi-head attention on 8 trn2 NeuronCores.

Problem (hardcoded): x[4, 2048, 768], w_attn[768, 2304], b_attn[2304],
w_proj[768, 768], b_proj[768]; H=6 heads, D=128 head dim; fp32 in/out.

Sharding: core c = 2*b + g handles batch b and head-group g (heads
3g..3g+2).  Each core computes Q/K/V projections for its 3 heads over the
full sequence, full causal attention for those heads, and a PARTIAL output
projection (w_proj rows of its heads).  The host sums the two partials per
batch and adds the bias terms.  No cross-core communication.

Bias algebra (host/device split):
  - b_q is added on device (affects scores).
  - b_k is dropped entirely: it shifts every score in a softmax row by the
    same constant, which cancels.
  - b_v is dropped on device: softmax rows sum to 1, so attn @ (v + b_v)
    = attn @ v + b_v; the constant (b_v @ w_proj + b_proj) is added on host.

Precision plan (HW rel err ~3e-3 vs the 2e-2 gate):
  - QKV projection runs as fp8e4m3 DoubleRow matmuls (2 contraction chunks
    per pass, 0.5 cycles/row = 4x fp32r throughput) using a hi+lo residual
    split of BOTH x and w_attn: x ~= xh + xl, w ~= wh + wl (each fp8), and
    q = xh@wh + xh@wl + xl@wh (the lo*lo term is ~1e-5 and dropped).  Nine
    DR matmuls replace six bf16 ones: 25% fewer PE cycles, same DMA bytes
    as bf16, all splits prepared on the host.  w_attn is pre-scaled by 32
    so its residual stays out of the fp8 subnormal floor; the 32x on q/k
    is folded into the exp scale and the 32x on v into w_proj (pre-divided
    by 32 on the host).
  - Everything else is bf16 operands with fp32 PSUM accumulation: scores,
    PV, output projection (1 cycle/row, same as fp32r, but half the SBUF
    and DMA).  fp8 anywhere else fails the error gate (outlier softmax
    rows: max|out|/rms ~ 32).

Softmax denominators: instead of 40 ones-matmuls per head (as many PE
cycles as PV itself), the exp'd score chunks are accumulated on the DVE
into a per-(group,head) fp16 tile (2-byte dtype = 2x DVE mode), and ONE
ones-matmul per (group,head) does the final 128-partition reduction,
replicating the denominators across partitions for the divide.

Causal masking: scores/exp are computed on full 512-row rectangles
(group t attends kv [0, 512(t+1))); the 4 diagonal kv chunks are fixed by
multiplying ex (bf16, SBUF) with 0/1 bf16 masks trimmed to the first
128(k+1) columns — 2-byte all-SBUF operands run in the DVE 2x mode,
~3x cheaper than the old additive -1e9 mask on fp32 PSUM.

Engine budget per core (cost model): PE ~104us (QKV 35 + scores 26 + PV 26
+ rowsum 3 + proj 15), Act ~75us (exp), DVE ~70us, Pool ~40us (copies).

Scheduling: inputs split into several DMAs ordered by first use; the
attention inner loop keeps a two-batch software-pipeline skew (PE is
in-order) so exp latency and the recip/divide/proj chain never stall PE.
"""

import math
from contextlib import ExitStack

import numpy as np
import ml_dtypes

import concourse.bacc as bacc
import concourse.bass as bass
import concourse.mybir as mybir
import concourse.tile as tile
from concourse import bass_utils

B, S, C = 4, 2048, 768
H, D = 6, 128
HL = 3          # heads per core
CK = C // 128   # 6 contraction chunks
CP = CK // 2    # 3 chunk pairs for DoubleRow
R = 512         # query rows per group
G = S // R      # 4 groups
N_CORES = 8
F32 = mybir.dt.float32
BF16 = mybir.dt.bfloat16
FP16 = mybir.dt.float16
FP8 = mybir.dt.float8e4
NP_FP8 = ml_dtypes.float8_e4m3
NP_BF16 = ml_dtypes.bfloat16
WS = 32.0                      # w_attn pre-scale (fp8 subnormal avoidance)
INV_SQRT_D = 1.0 / math.sqrt(D)
EXP_SCALE = INV_SQRT_D / (WS * WS)
MASKW = 128                    # one shared [128,128] causal triangle mask
DR = mybir.MatmulPerfMode.DoubleRow


def _emit(ctx: ExitStack, tc: tile.TileContext, xa, wav, wqk,
          bq, mask01, ones, wp, outT):
    nc = tc.nc

    singles = ctx.enter_context(tc.tile_pool(name="singles", bufs=1))
    expool = ctx.enter_context(tc.tile_pool(name="expool", bufs=8))
    aopool = ctx.enter_context(tc.tile_pool(name="aopool", bufs=2))
    otpool = ctx.enter_context(tc.tile_pool(name="otpool", bufs=4))
    rspool = ctx.enter_context(tc.tile_pool(name="rspool", bufs=3))
    accpool = ctx.enter_context(tc.tile_pool(name="accpool", bufs=3))
    psum = ctx.enter_context(tc.tile_pool(name="psum", bufs=2, space="PSUM"))

    # ---- resident loads, split + ordered by first use ----
    xa_sb = singles.tile([128, G, 2, CK, R], FP8)
    wav_sb = singles.tile([128, 2, CK, HL * D], FP8)
    wqk_sb = singles.tile([128, 2, CK, 2 * HL * D], FP8)
    bq_sb = singles.tile([128, HL], F32)
    mask_sb = singles.tile([128, MASKW], BF16)
    ones_sb = singles.tile([128, 128], FP16)
    wp_sb = singles.tile([128, HL, C], BF16)

    def load_half(n, hi):
        w, half = 2 * CK * R, CK * R
        nc.sync.dma_start(
            xa_sb[:, n, hi],
            xa[:, n * w + hi * half:n * w + (hi + 1) * half]
            .rearrange("p (c s) -> p c s", c=CK))

    def load_whalf(w_sb, w_dram, hi, width):
        nc.sync.dma_start(
            w_sb[:, hi],
            w_dram[:, hi * CK * width:(hi + 1) * CK * width]
            .rearrange("p (c n) -> p c n", c=CK))

    # ordered by first use: V tiles run hi*hi first (xa0h+wavh), then
    # hi*lo (wavl), then lo*hi (xa0l); QK tiles follow (wqk hi, lo)
    load_half(0, 0)
    load_whalf(wav_sb, wav, 0, HL * D)
    load_whalf(wav_sb, wav, 1, HL * D)
    load_half(0, 1)
    load_whalf(wqk_sb, wqk, 0, 2 * HL * D)
    load_whalf(wqk_sb, wqk, 1, 2 * HL * D)
    nc.sync.dma_start(bq_sb, bq)
    nc.sync.dma_start(mask_sb, mask01)
    nc.sync.dma_start(ones_sb, ones)
    for n in range(1, G):
        load_half(n, 0)
        load_half(n, 1)
    nc.sync.dma_start(wp_sb, wp.rearrange("p (f n) -> p f n", f=HL))

    # ---- QKV projections: 9 fp8 DoubleRow matmuls per output tile
    # (3 chunk pairs x 3 hi/lo cross terms), interleaved per x token-chunk
    # so PE work tracks DMA arrival.
    V_sb = singles.tile([128, S // 128, HL * D], BF16)
    qkT_sb = singles.tile([128, 2 * HL, S], BF16)

    def v_terms(ps, n, r, terms, start, stop):
        idx = 0
        nt = len(terms) * CP
        for xi, wi in terms:
            for cp in range(CP):
                nc.tensor.matmul(
                    ps[:, :HL * D],
                    lhsT=xa_sb[:, n, xi, 2 * cp:2 * cp + 2,
                               (r % 4) * 128:(r % 4 + 1) * 128],
                    rhs=wav_sb[:, wi, 2 * cp:2 * cp + 2, :],
                    start=(start and idx == 0),
                    stop=(stop and idx == nt - 1),
                    perf_mode=DR,
                )
                idx += 1

    def v_tile(n, r, tag):
        ps = psum.tile([128, R], F32, tag=tag)
        v_terms(ps, n, r, ((0, 0), (0, 1), (1, 0)), True, True)
        nc.scalar.copy(V_sb[:, r, :], ps[:, :HL * D])

    def v_pair_hi_first(n, ra, rb):
        # hi-dependent terms of two tiles first, lo-x terms after: lets PE
        # start before the xa-lo DMA lands (prologue only)
        psa = psum.tile([128, R], F32, tag="st")
        v_terms(psa, n, ra, ((0, 0), (0, 1)), True, False)
        psb = psum.tile([128, R], F32, tag="st")
        v_terms(psb, n, rb, ((0, 0), (0, 1)), True, False)
        v_terms(psa, n, ra, ((1, 0),), False, True)
        nc.scalar.copy(V_sb[:, ra, :], psa[:, :HL * D])
        v_terms(psb, n, rb, ((1, 0),), False, True)
        nc.scalar.copy(V_sb[:, rb, :], psb[:, :HL * D])

    def qk_tile(n, f, tag):
        ps = psum.tile([128, R], F32, tag=tag)
        idx = 0
        for wi, xi in ((0, 0), (0, 1), (1, 0)):
            for cp in range(CP):
                nc.tensor.matmul(
                    ps,
                    lhsT=wqk_sb[:, wi, 2 * cp:2 * cp + 2, f * 128:(f + 1) * 128],
                    rhs=xa_sb[:, n, xi, 2 * cp:2 * cp + 2, :],
                    start=(idx == 0),
                    stop=(idx == 3 * CP - 1),
                    perf_mode=DR,
                )
                idx += 1
        if f < HL:
            nc.scalar.add(qkT_sb[:, f, n * R:(n + 1) * R], ps, bq_sb[:, f:f + 1])
        else:
            nc.scalar.copy(qkT_sb[:, f, n * R:(n + 1) * R], ps)

    # groups 0-2 inline; group 3 becomes filler work popped inside the
    # attention pipeline (t0-t2 never read group-3 Q/K/V, and their
    # per-head stalls are exactly PE-sized holes)
    for n in range(G - 1):
        for r in range(4 * n, 4 * n + 4):
            v_tile(n, r, "st")
        for f in range(2 * HL):
            qk_tile(n, f, "st")
    qkv_fill = [lambda r=r: v_tile(3, r, "rs") for r in range(12, 16)]
    qkv_fill += [lambda f=f: qk_tile(3, f, "rs") for f in range(2 * HL)]

    # ---- attention + output projection, software-pipelined ----
    pending = []
    proj_queue = []

    def push(fn):
        pending.append(fn)
        while len(pending) > 2:
            pending.pop(0)()

    def pop_fill(k):
        for _ in range(k):
            if qkv_fill:
                qkv_fill.pop(0)()
            elif proj_queue:
                proj_queue.pop(0)()

    def drain():
        while pending:
            pending.pop(0)()
        while proj_queue:
            proj_queue.pop(0)()

    # Diagonal kv chunk k (k = j-(nk-4)) only attends query columns
    # [128k, 512): scores/exp/PV/acc all run on that suffix, and the causal
    # triangle inside its first 128 columns is fixed by ONE shared [128,128]
    # 0/1 mask.  Chunk j=0 is always full-width, so the PV psum start=True
    # initializes the whole [128, R] region.
    for t in (0, 1, 2, 3):
        if t == 3:
            while qkv_fill:  # t=3 reads group-3 Q/K/V: finish any leftovers
                qkv_fill.pop(0)()
        rows = t * R
        nk = 4 * (t + 1)

        def off(j, nk=nk):
            return 128 * (j - (nk - 4)) if j >= nk - 4 else 0

        ao = aopool.tile([128, HL, R], BF16, tag="ao")
        for h in range(HL):
            pv = psum.tile([128, R], F32, tag="pv")
            acc = accpool.tile([128, R], FP16, tag="acc")
            for jb in range(nk // 2):
                # late fillers land where the Act lag peaks; deferred
                # QKV-g3 tiles first, then queued proj blocks
                if jb >= nk // 2 - (3 if t == 3 else 2):
                    pop_fill(1)
                if jb == nk // 2 - 1 and t != 3:
                    pop_fill(1)
                o0, o1 = off(2 * jb), off(2 * jb + 1)
                st = psum.tile([128, 2, R], F32, tag="st")
                for u, o in ((0, o0), (1, o1)):
                    j = 2 * jb + u
                    nc.tensor.matmul(
                        st[:, u, o:],
                        lhsT=qkT_sb[:, HL + h, j * 128:(j + 1) * 128],
                        rhs=qkT_sb[:, h, rows + o:rows + R],
                        start=True,
                        stop=True,
                    )
                ex = expool.tile([128, 2, R], BF16, tag="ex")
                # one exp over the common suffix, plus chunk0's head columns
                nc.scalar.activation(
                    ex[:, :, o1:], st[:, :, o1:],
                    mybir.ActivationFunctionType.Exp, scale=EXP_SCALE,
                )
                if o1 > o0:
                    nc.scalar.activation(
                        ex[:, 0, o0:o1], st[:, 0, o0:o1],
                        mybir.ActivationFunctionType.Exp, scale=EXP_SCALE,
                    )
                # causal fix-up: 0/1 triangle mult on each diagonal chunk's
                # first 128 valid columns (bf16 all-SBUF = DVE 2x mode)
                for u, o in ((0, o0), (1, o1)):
                    j = 2 * jb + u
                    if j >= nk - 4:
                        nc.vector.tensor_tensor(
                            ex[:, u, o:o + 128], ex[:, u, o:o + 128],
                            mask_sb, mybir.AluOpType.mult,
                        )
                # fp16 denominator accumulation on DVE (2x mode): tmp is
                # independent per jb; only the acc += tmp adds are chained
                if jb == 0:
                    if o1 == 0:
                        nc.vector.tensor_tensor(
                            acc, ex[:, 0, :], ex[:, 1, :], mybir.AluOpType.add)
                    else:  # t == 0: ragged first pair
                        nc.vector.tensor_copy(acc, ex[:, 0, :])
                        nc.vector.tensor_tensor(
                            acc[:, o1:], acc[:, o1:], ex[:, 1, o1:],
                            mybir.AluOpType.add)
                elif o0 == o1 == 0:
                    tmp = accpool.tile([128, R], FP16, tag="tmp")
                    nc.vector.tensor_tensor(
                        tmp, ex[:, 0, :], ex[:, 1, :], mybir.AluOpType.add)
                    nc.vector.tensor_tensor(acc, acc, tmp, mybir.AluOpType.add)
                else:  # diagonal pair: two direct suffix adds
                    nc.vector.tensor_tensor(
                        acc[:, o0:], acc[:, o0:], ex[:, 0, o0:],
                        mybir.AluOpType.add)
                    nc.vector.tensor_tensor(
                        acc[:, o1:], acc[:, o1:], ex[:, 1, o1:],
                        mybir.AluOpType.add)

                def consume(jb=jb, h=h, pv=pv, ex=ex, nk=nk, o0=o0, o1=o1):
                    for u, o in ((0, o0), (1, o1)):
                        j = 2 * jb + u
                        nc.tensor.matmul(
                            pv[:, o:],
                            lhsT=V_sb[:, j, h * D:(h + 1) * D],
                            rhs=ex[:, u, o:],
                            start=(j == 0),
                            stop=(j == nk - 1),
                        )

                push(consume)

            def finalize(h=h, t=t, pv=pv, acc=acc, ao=ao):
                rs = psum.tile([128, R], F32, tag="rs")
                nc.tensor.matmul(rs, lhsT=ones_sb, rhs=acc, start=True, stop=True)
                rsr = rspool.tile([128, R], F32, tag="rsr")
                nc.vector.reciprocal(rsr, rs)
                nc.vector.tensor_tensor(
                    ao[:, h, :], pv, rsr, mybir.AluOpType.mult)
                if h == HL - 1:
                    proj_queue.extend(
                        _proj_obs(nc, psum, otpool, wp_sb, ao, outT, t))

            push(finalize)
    drain()


def _proj_obs(nc, psum, otpool, wp_sb, ao, outT, t):
    rows = slice(t * R, (t + 1) * R)
    last = t == G - 1

    def one(ob):
        ps = psum.tile([128, R], mybir.dt.float32, tag="pv")
        for fc in range(HL):
            nc.tensor.matmul(
                ps,
                lhsT=wp_sb[:, fc, ob * 128:(ob + 1) * 128],
                rhs=ao[:, fc, :],
                start=(fc == 0),
                stop=(fc == HL - 1),
            )
        ot = otpool.tile([128, R], BF16, tag="ot")
        # last group's copies drain after all other work: split across
        # Act (idle by then) and DVE so the tail halves
        if last and ob % 2 == 0:
            nc.scalar.copy(ot, ps)
        else:
            nc.vector.tensor_copy(ot, ps)
        nc.sync.dma_start(outT[ob * 128:(ob + 1) * 128, rows], ot)

    return [lambda ob=ob: one(ob) for ob in range(C // 128)]


_CACHED = None


def _build():
    global _CACHED
    if _CACHED is not None:
        return _CACHED
    nc = bacc.Bacc(
        "TRN2",
        target_bir_lowering=False,
        debug=False,
        enable_asserts=False,
        num_devices=N_CORES,
    )
    xa = nc.dram_tensor("xa", [128, G * 2 * CK * R], FP8, kind="ExternalInput").ap()
    wav = nc.dram_tensor("wav", [128, 2 * CK * HL * D], FP8, kind="ExternalInput").ap()
    wqk = nc.dram_tensor("wqk", [128, 2 * CK * 2 * HL * D], FP8, kind="ExternalInput").ap()
    bq = nc.dram_tensor("bq", [128, HL], F32, kind="ExternalInput").ap()
    mask01 = nc.dram_tensor("mask01", [128, MASKW], BF16, kind="ExternalInput").ap()
    ones = nc.dram_tensor("ones", [128, 128], FP16, kind="ExternalInput").ap()
    wp = nc.dram_tensor("wp", [128, HL * C], BF16, kind="ExternalInput").ap()
    outT = nc.dram_tensor("outT", [C, S], BF16, kind="ExternalOutput").ap()
    with tile.TileContext(nc) as tc, ExitStack() as ctx:
        _emit(ctx, tc, xa, wav, wqk, bq, mask01, ones, wp, outT)
    nc.compile()
    _CACHED = nc
    return nc


def _pmajor(a2d):
    """[n*128, w] -> [128, n*w]: partition-major shuffle for one-DMA loads."""
    n = a2d.shape[0] // 128
    w = a2d.shape[1]
    return np.ascontiguousarray(
        a2d.reshape(n, 128, w).transpose(1, 0, 2).reshape(128, n * w))


def _hilo(a):
    """fp32 array -> (hi, lo) fp8e4m3 pair with hi + lo ~= a."""
    hi = a.astype(NP_FP8)
    lo = (a - hi.astype(np.float32)).astype(NP_FP8)
    return hi, lo


def _masks01():
    """Shared 0/1 bf16 causal triangle: mask[p, q] = 1 iff q >= p."""
    p = np.arange(128)[:, None]
    q = np.arange(128)[None, :]
    return (q >= p).astype(NP_BF16)  # [128, 128]


def shard_inputs(x, w_attn, b_attn, w_proj):
    """Per-core input dicts for cores 0..7 (core = 2*batch + head_group)."""
    masks = _masks01()
    ones = np.ones((128, 128), np.float16)
    xs = []
    for b in range(B):
        xT = np.ascontiguousarray(x[b].T)  # [768, 2048]
        xh, xl = _hilo(xT)
        # per group: [hi block | lo block], each chunk-major
        xs.append(np.concatenate(
            [_pmajor(xq[:, n * CK * 0 + n * R:(n + 1) * R])
             for n in range(G) for xq in (xh, xl)], axis=1))
    in_maps = []
    for c in range(N_CORES):
        b, g = divmod(c, 2)
        lo, hi = g * HL * D, (g + 1) * HL * D
        wav = w_attn[:, 2 * C + lo:2 * C + hi] * WS
        wqk = np.concatenate(
            [w_attn[:, lo:hi], w_attn[:, C + lo:C + hi]], axis=1) * WS
        wavh, wavl = _hilo(wav)
        wqkh, wqkl = _hilo(wqk)
        bqv = (WS * b_attn[lo:hi]).astype(np.float32).reshape(HL, 128).T
        in_maps.append({
            "xa": xs[b],
            "wav": np.concatenate([_pmajor(wavh), _pmajor(wavl)], axis=1),
            "wqk": np.concatenate([_pmajor(wqkh), _pmajor(wqkl)], axis=1),
            "bq": np.ascontiguousarray(bqv),
            "mask01": masks,
            "ones": ones,
            "wp": _pmajor((w_proj[lo:hi, :] / WS).astype(NP_BF16)),
        })
    return in_maps


def combine_outputs(parts, b_attn, w_proj, b_proj):
    """parts[c] = outT partial [768, 2048] (bf16) from core c."""
    bias = b_attn[2 * C:].astype(np.float64) @ w_proj.astype(np.float64) + b_proj
    out = np.empty((B, S, C), np.float32)
    for b in range(B):
        acc = parts[2 * b].astype(np.float32) + parts[2 * b + 1].astype(np.float32)
        out[b] = acc.T + bias.astype(np.float32)[None, :]
    return out


def kernel(x, w_attn, b_attn, w_proj, b_proj, **run_kwargs):
    x = np.asarray(x, np.float32)
    w_attn = np.asarray(w_attn, np.float32)
    b_attn = np.asarray(b_attn, np.float32)
    w_proj = np.asarray(w_proj, np.float32)
    b_proj = np.asarray(b_proj, np.float32)

    nc = _build()
    in_maps = shard_inputs(x, w_attn, b_attn, w_proj)
    res = bass_utils.run_bass_kernel_spmd(
        nc, in_maps, core_ids=list(range(N_CORES)), **run_kwargs
    )
    parts = [r["outT"] for r in res.results]
    out = combine_outputs(parts, b_attn, w_proj, b_proj)
    kernel.last_results = res
    return out


# revision 60
# speedup vs baseline: 1.5256x; 1.0021x over previous
"""Causal multi-head attention on 8 trn2 NeuronCores.

Problem (hardcoded): x[4, 2048, 768], w_attn[768, 2304], b_attn[2304],
w_proj[768, 768], b_proj[768]; H=6 heads, D=128 head dim; fp32 in/out.

Sharding: core c = 2*b + g handles batch b and head-group g (heads
3g..3g+2).  Each core computes Q/K/V projections for its 3 heads over the
full sequence, full causal attention for those heads, and a PARTIAL output
projection (w_proj rows of its heads).  The host sums the two partials per
batch and adds the bias terms.  No cross-core communication.

Bias algebra (host/device split):
  - b_q is added on device (affects scores).
  - b_k is dropped entirely: it shifts every score in a softmax row by the
    same constant, which cancels.
  - b_v is dropped on device: softmax rows sum to 1, so attn @ (v + b_v)
    = attn @ v + b_v; the constant (b_v @ w_proj + b_proj) is added on host.

Precision plan (HW rel err ~3e-3 vs the 2e-2 gate):
  - QKV projection runs as fp8e4m3 DoubleRow matmuls (2 contraction chunks
    per pass, 0.5 cycles/row = 4x fp32r throughput) using a hi+lo residual
    split of BOTH x and w_attn: x ~= xh + xl, w ~= wh + wl (each fp8), and
    q = xh@wh + xh@wl + xl@wh (the lo*lo term is ~1e-5 and dropped).  Nine
    DR matmuls replace six bf16 ones: 25% fewer PE cycles, same DMA bytes
    as bf16, all splits prepared on the host.  w_attn is pre-scaled by 32
    so its residual stays out of the fp8 subnormal floor; the 32x on q/k
    is folded into the exp scale and the 32x on v into w_proj (pre-divided
    by 32 on the host).
  - Everything else is bf16 operands with fp32 PSUM accumulation: scores,
    PV, output projection (1 cycle/row, same as fp32r, but half the SBUF
    and DMA).  fp8 anywhere else fails the error gate (outlier softmax
    rows: max|out|/rms ~ 32).

Softmax denominators: instead of 40 ones-matmuls per head (as many PE
cycles as PV itself), the exp'd score chunks are accumulated on the DVE
into a per-(group,head) fp16 tile (2-byte dtype = 2x DVE mode), and ONE
ones-matmul per (group,head) does the final 128-partition reduction,
replicating the denominators across partitions for the divide.

Causal structure: group t attends kv [0, 512(t+1)); diagonal kv chunk k
(k = j-(nk-4)) only attends query columns [128k, 512), so scores/exp/PV/
denominator-accumulation all run on that suffix (-15% attention work),
and the triangle inside its first 128 valid columns is fixed by ONE
shared [128,128] 0/1 bf16 mask multiplied into ex (DVE 2x mode).

Engine budget per core (cost model): PE ~96us busy (QKV 35 + scores 22 +
PV 22 + rowsum 2.6 + proj 15), Act ~85us (exp + qkv-psum copies), DVE
~67us, total 114.8us.

Scheduling: inputs split into DMAs ordered by first use; the attention
inner loop keeps a two-batch software-pipeline skew (PE is in-order) so
exp latency never stalls PE; QKV group 3 (only needed by attention group
t=3) and the output-projection blocks are deferred into filler queues
popped late in each attention head, exactly where the slower Act engine
(exp: 1028ns vs PE 853ns per kv-chunk pair) falls behind.
"""

import math
from contextlib import ExitStack

import numpy as np
import ml_dtypes

import concourse.bacc as bacc
import concourse.bass as bass
import concourse.mybir as mybir
import concourse.tile as tile
from concourse import bass_utils

B, S, C = 4, 2048, 768
H, D = 6, 128
HL = 3          # heads per core
CK = C // 128   # 6 contraction chunks
CP = CK // 2    # 3 chunk pairs for DoubleRow
R = 512         # query rows per group
G = S // R      # 4 groups
N_CORES = 8
F32 = mybir.dt.float32
BF16 = mybir.dt.bfloat16
FP16 = mybir.dt.float16
FP8 = mybir.dt.float8e4
NP_FP8 = ml_dtypes.float8_e4m3
NP_BF16 = ml_dtypes.bfloat16
WS = 32.0                      # w_attn pre-scale (fp8 subnormal avoidance)
INV_SQRT_D = 1.0 / math.sqrt(D)
EXP_SCALE = INV_SQRT_D / (WS * WS)
MASKW = 128                    # one shared [128,128] causal triangle mask
DR = mybir.MatmulPerfMode.DoubleRow


def _emit(ctx: ExitStack, tc: tile.TileContext, xa, wav, wqk,
          bq, mask01, ones, wp, outT):
    nc = tc.nc

    singles = ctx.enter_context(tc.tile_pool(name="singles", bufs=1))
    expool = ctx.enter_context(tc.tile_pool(name="expool", bufs=8))
    aopool = ctx.enter_context(tc.tile_pool(name="aopool", bufs=2))
    otpool = ctx.enter_context(tc.tile_pool(name="otpool", bufs=4))
    rspool = ctx.enter_context(tc.tile_pool(name="rspool", bufs=3))
    accpool = ctx.enter_context(tc.tile_pool(name="accpool", bufs=3))
    psum = ctx.enter_context(tc.tile_pool(name="psum", bufs=2, space="PSUM"))

    # ---- resident loads, split + ordered by first use ----
    xa_sb = singles.tile([128, G, 2, CK, R], FP8)
    wav_sb = singles.tile([128, 2, CK, HL * D], FP8)
    wqk_sb = singles.tile([128, 2, CK, 2 * HL * D], FP8)
    bq_sb = singles.tile([128, HL], F32)
    mask_sb = singles.tile([128, MASKW], BF16)
    ones_sb = singles.tile([128, 128], FP16)
    wp_sb = singles.tile([128, HL, C], BF16)

    def load_half(n, hi):
        w, half = 2 * CK * R, CK * R
        nc.sync.dma_start(
            xa_sb[:, n, hi],
            xa[:, n * w + hi * half:n * w + (hi + 1) * half]
            .rearrange("p (c s) -> p c s", c=CK))

    def load_whalf(w_sb, w_dram, hi, width):
        nc.sync.dma_start(
            w_sb[:, hi],
            w_dram[:, hi * CK * width:(hi + 1) * CK * width]
            .rearrange("p (c n) -> p c n", c=CK))

    # ordered by first use: V tiles run hi*hi first (xa0h+wavh), then
    # hi*lo (wavl), then lo*hi (xa0l); QK tiles follow (wqk hi, lo)
    load_half(0, 0)
    load_whalf(wav_sb, wav, 0, HL * D)
    load_whalf(wav_sb, wav, 1, HL * D)
    load_half(0, 1)
    load_whalf(wqk_sb, wqk, 0, 2 * HL * D)
    load_whalf(wqk_sb, wqk, 1, 2 * HL * D)
    nc.sync.dma_start(bq_sb, bq)
    nc.sync.dma_start(mask_sb, mask01)
    nc.sync.dma_start(ones_sb, ones)
    for n in range(1, G):
        load_half(n, 0)
        load_half(n, 1)
    nc.sync.dma_start(wp_sb, wp.rearrange("p (f n) -> p f n", f=HL))

    # ---- QKV projections: 9 fp8 DoubleRow matmuls per output tile
    # (3 chunk pairs x 3 hi/lo cross terms), interleaved per x token-chunk
    # so PE work tracks DMA arrival.
    V_sb = singles.tile([128, S // 128, HL * D], BF16)
    qkT_sb = singles.tile([128, 2 * HL, S], BF16)

    def v_terms(ps, n, r, terms, start, stop):
        idx = 0
        nt = len(terms) * CP
        for xi, wi in terms:
            for cp in range(CP):
                nc.tensor.matmul(
                    ps[:, :HL * D],
                    lhsT=xa_sb[:, n, xi, 2 * cp:2 * cp + 2,
                               (r % 4) * 128:(r % 4 + 1) * 128],
                    rhs=wav_sb[:, wi, 2 * cp:2 * cp + 2, :],
                    start=(start and idx == 0),
                    stop=(stop and idx == nt - 1),
                    perf_mode=DR,
                )
                idx += 1

    def v_tile(n, r, tag):
        ps = psum.tile([128, R], F32, tag=tag)
        v_terms(ps, n, r, ((0, 0), (0, 1), (1, 0)), True, True)
        if n == 3:  # filler tile mid-attention: keep Act (exp) free
            nc.vector.tensor_copy(V_sb[:, r, :], ps[:, :HL * D])
        else:
            nc.scalar.copy(V_sb[:, r, :], ps[:, :HL * D])



    def qk_tile(n, f, tag):
        ps = psum.tile([128, R], F32, tag=tag)
        idx = 0
        for wi, xi in ((0, 0), (0, 1), (1, 0)):
            for cp in range(CP):
                nc.tensor.matmul(
                    ps,
                    lhsT=wqk_sb[:, wi, 2 * cp:2 * cp + 2, f * 128:(f + 1) * 128],
                    rhs=xa_sb[:, n, xi, 2 * cp:2 * cp + 2, :],
                    start=(idx == 0),
                    stop=(idx == 3 * CP - 1),
                    perf_mode=DR,
                )
                idx += 1
        dst = qkT_sb[:, f, n * R:(n + 1) * R]
        if f >= HL:  # k: plain copy
            if n == 3:  # filler tile mid-attention: keep Act (exp) free
                nc.vector.tensor_copy(dst, ps)
            else:
                nc.scalar.copy(dst, ps)
        elif n == 3:
            nc.vector.tensor_scalar_add(dst, ps, bq_sb[:, f:f + 1])
        else:
            nc.scalar.add(dst, ps, bq_sb[:, f:f + 1])

    # groups 0-2 inline; group 3 becomes filler work popped inside the
    # attention pipeline (t0-t2 never read group-3 Q/K/V, and their
    # per-head stalls are exactly PE-sized holes)
    for n in range(G - 1):
        for r in range(4 * n, 4 * n + 4):
            v_tile(n, r, "st")
        for f in range(2 * HL):
            qk_tile(n, f, "st")
    qkv_fill = [lambda r=r: v_tile(3, r, "rs") for r in range(12, 16)]
    qkv_fill += [lambda f=f: qk_tile(3, f, "rs") for f in range(2 * HL)]

    # ---- attention + output projection, software-pipelined ----
    pending = []
    proj_queue = []

    def push(fn):
        pending.append(fn)
        while len(pending) > 2:
            pending.pop(0)()

    def pop_fill(k):
        for _ in range(k):
            if qkv_fill:
                qkv_fill.pop(0)()
            elif proj_queue:
                proj_queue.pop(0)()

    def drain():
        while pending:
            pending.pop(0)()
        while proj_queue:
            proj_queue.pop(0)()

    # Diagonal kv chunk k (k = j-(nk-4)) only attends query columns
    # [128k, 512): scores/exp/PV/acc all run on that suffix, and the causal
    # triangle inside its first 128 columns is fixed by ONE shared [128,128]
    # 0/1 mask.  Chunk j=0 is always full-width, so the PV psum start=True
    # initializes the whole [128, R] region.
    for t in (0, 1, 2, 3):
        if t == 3:
            while qkv_fill:  # t=3 reads group-3 Q/K/V: finish any leftovers
                qkv_fill.pop(0)()
        rows = t * R
        nk = 4 * (t + 1)

        def off(j, nk=nk):
            return 128 * (j - (nk - 4)) if j >= nk - 4 else 0

        ao = aopool.tile([128, HL, R], BF16, tag="ao")
        for h in range(HL):
            pv = psum.tile([128, R], F32, tag="pv")
            acc = accpool.tile([128, R], FP16, tag="acc")
            for jb in range(nk // 2):
                # late fillers land where the Act lag peaks; deferred
                # QKV-g3 tiles first, then queued proj blocks
                if jb >= nk // 2 - (3 if t == 3 else 2):
                    pop_fill(1)
                if jb == nk // 2 - 1 and t != 3:
                    pop_fill(1)
                o0, o1 = off(2 * jb), off(2 * jb + 1)
                st = psum.tile([128, 2, R], F32, tag="st")
                for u, o in ((0, o0), (1, o1)):
                    j = 2 * jb + u
                    nc.tensor.matmul(
                        st[:, u, o:],
                        lhsT=qkT_sb[:, HL + h, j * 128:(j + 1) * 128],
                        rhs=qkT_sb[:, h, rows + o:rows + R],
                        start=True,
                        stop=True,
                    )
                ex = expool.tile([128, 2, R], BF16, tag="ex")
                # one exp over the common suffix, plus chunk0's head columns
                nc.scalar.activation(
                    ex[:, :, o1:], st[:, :, o1:],
                    mybir.ActivationFunctionType.Exp, scale=EXP_SCALE,
                )
                if o1 > o0:
                    nc.scalar.activation(
                        ex[:, 0, o0:o1], st[:, 0, o0:o1],
                        mybir.ActivationFunctionType.Exp, scale=EXP_SCALE,
                    )
                # causal fix-up: 0/1 triangle mult on each diagonal chunk's
                # first 128 valid columns (bf16 all-SBUF = DVE 2x mode)
                for u, o in ((0, o0), (1, o1)):
                    j = 2 * jb + u
                    if j >= nk - 4:
                        nc.vector.tensor_tensor(
                            ex[:, u, o:o + 128], ex[:, u, o:o + 128],
                            mask_sb, mybir.AluOpType.mult,
                        )
                # fp16 denominator accumulation on DVE (2x mode): tmp is
                # independent per jb; only the acc += tmp adds are chained
                if jb == 0:
                    if o1 == 0:
                        nc.vector.tensor_tensor(
                            acc, ex[:, 0, :], ex[:, 1, :], mybir.AluOpType.add)
                    else:  # t == 0: ragged first pair
                        nc.vector.tensor_copy(acc, ex[:, 0, :])
                        nc.vector.tensor_tensor(
                            acc[:, o1:], acc[:, o1:], ex[:, 1, o1:],
                            mybir.AluOpType.add)
                elif o0 == o1 == 0:
                    tmp = accpool.tile([128, R], FP16, tag="tmp")
                    nc.vector.tensor_tensor(
                        tmp, ex[:, 0, :], ex[:, 1, :], mybir.AluOpType.add)
                    nc.vector.tensor_tensor(acc, acc, tmp, mybir.AluOpType.add)
                else:  # diagonal pair: two direct suffix adds
                    nc.vector.tensor_tensor(
                        acc[:, o0:], acc[:, o0:], ex[:, 0, o0:],
                        mybir.AluOpType.add)
                    nc.vector.tensor_tensor(
                        acc[:, o1:], acc[:, o1:], ex[:, 1, o1:],
                        mybir.AluOpType.add)

                def consume(jb=jb, h=h, pv=pv, ex=ex, nk=nk, o0=o0, o1=o1):
                    for u, o in ((0, o0), (1, o1)):
                        j = 2 * jb + u
                        nc.tensor.matmul(
                            pv[:, o:],
                            lhsT=V_sb[:, j, h * D:(h + 1) * D],
                            rhs=ex[:, u, o:],
                            start=(j == 0),
                            stop=(j == nk - 1),
                        )

                push(consume)

            def finalize(h=h, t=t, pv=pv, acc=acc, ao=ao):
                rs = psum.tile([128, R], F32, tag="rs")
                nc.tensor.matmul(rs, lhsT=ones_sb, rhs=acc, start=True, stop=True)
                rsr = rspool.tile([128, R], F32, tag="rsr")
                nc.vector.reciprocal(rsr, rs)
                nc.vector.tensor_tensor(
                    ao[:, h, :], pv, rsr, mybir.AluOpType.mult)
                if h == HL - 1:
                    proj_queue.extend(
                        _proj_obs(nc, psum, otpool, wp_sb, ao, outT, t))

            push(finalize)
    drain()


def _proj_obs(nc, psum, otpool, wp_sb, ao, outT, t):
    rows = slice(t * R, (t + 1) * R)
    last = t == G - 1

    def one(ob):
        ps = psum.tile([128, R], mybir.dt.float32, tag="pv")
        for fc in range(HL):
            nc.tensor.matmul(
                ps,
                lhsT=wp_sb[:, fc, ob * 128:(ob + 1) * 128],
                rhs=ao[:, fc, :],
                start=(fc == 0),
                stop=(fc == HL - 1),
            )
        ot = otpool.tile([128, R], BF16, tag="ot")
        # last group's copies drain after all other work: split across
        # Act (idle by then) and DVE so the tail halves
        if last and ob % 2 == 0:
            nc.scalar.copy(ot, ps)
        else:
            nc.vector.tensor_copy(ot, ps)
        nc.sync.dma_start(outT[ob * 128:(ob + 1) * 128, rows], ot)

    return [lambda ob=ob: one(ob) for ob in range(C // 128)]


_CACHED = None


def _build():
    global _CACHED
    if _CACHED is not None:
        return _CACHED
    nc = bacc.Bacc(
        "TRN2",
        target_bir_lowering=False,
        debug=False,
        enable_asserts=False,
        num_devices=N_CORES,
    )
    xa = nc.dram_tensor("xa", [128, G * 2 * CK * R], FP8, kind="ExternalInput").ap()
    wav = nc.dram_tensor("wav", [128, 2 * CK * HL * D], FP8, kind="ExternalInput").ap()
    wqk = nc.dram_tensor("wqk", [128, 2 * CK * 2 * HL * D], FP8, kind="ExternalInput").ap()
    bq = nc.dram_tensor("bq", [128, HL], F32, kind="ExternalInput").ap()
    mask01 = nc.dram_tensor("mask01", [128, MASKW], BF16, kind="ExternalInput").ap()
    ones = nc.dram_tensor("ones", [128, 128], FP16, kind="ExternalInput").ap()
    wp = nc.dram_tensor("wp", [128, HL * C], BF16, kind="ExternalInput").ap()
    outT = nc.dram_tensor("outT", [C, S], BF16, kind="ExternalOutput").ap()
    with tile.TileContext(nc) as tc, ExitStack() as ctx:
        _emit(ctx, tc, xa, wav, wqk, bq, mask01, ones, wp, outT)
    nc.compile()
    _CACHED = nc
    return nc


def _pmajor(a2d):
    """[n*128, w] -> [128, n*w]: partition-major shuffle for one-DMA loads."""
    n = a2d.shape[0] // 128
    w = a2d.shape[1]
    return np.ascontiguousarray(
        a2d.reshape(n, 128, w).transpose(1, 0, 2).reshape(128, n * w))


def _hilo(a):
    """fp32 array -> (hi, lo) fp8e4m3 pair with hi + lo ~= a."""
    hi = a.astype(NP_FP8)
    lo = (a - hi.astype(np.float32)).astype(NP_FP8)
    return hi, lo


def _masks01():
    """Shared 0/1 bf16 causal triangle: mask[p, q] = 1 iff q >= p."""
    p = np.arange(128)[:, None]
    q = np.arange(128)[None, :]
    return (q >= p).astype(NP_BF16)  # [128, 128]


def shard_inputs(x, w_attn, b_attn, w_proj):
    """Per-core input dicts for cores 0..7 (core = 2*batch + head_group)."""
    masks = _masks01()
    ones = np.ones((128, 128), np.float16)
    xs = []
    for b in range(B):
        xT = np.ascontiguousarray(x[b].T)  # [768, 2048]
        xh, xl = _hilo(xT)
        # per group: [hi block | lo block], each chunk-major
        xs.append(np.concatenate(
            [_pmajor(xq[:, n * CK * 0 + n * R:(n + 1) * R])
             for n in range(G) for xq in (xh, xl)], axis=1))
    in_maps = []
    for c in range(N_CORES):
        b, g = divmod(c, 2)
        lo, hi = g * HL * D, (g + 1) * HL * D
        wav = w_attn[:, 2 * C + lo:2 * C + hi] * WS
        wqk = np.concatenate(
            [w_attn[:, lo:hi], w_attn[:, C + lo:C + hi]], axis=1) * WS
        wavh, wavl = _hilo(wav)
        wqkh, wqkl = _hilo(wqk)
        bqv = (WS * b_attn[lo:hi]).astype(np.float32).reshape(HL, 128).T
        in_maps.append({
            "xa": xs[b],
            "wav": np.concatenate([_pmajor(wavh), _pmajor(wavl)], axis=1),
            "wqk": np.concatenate([_pmajor(wqkh), _pmajor(wqkl)], axis=1),
            "bq": np.ascontiguousarray(bqv),
            "mask01": masks,
            "ones": ones,
            "wp": _pmajor((w_proj[lo:hi, :] / WS).astype(NP_BF16)),
        })
    return in_maps


def combine_outputs(parts, b_attn, w_proj, b_proj):
    """parts[c] = outT partial [768, 2048] (bf16) from core c."""
    bias = b_attn[2 * C:].astype(np.float64) @ w_proj.astype(np.float64) + b_proj
    out = np.empty((B, S, C), np.float32)
    for b in range(B):
        acc = parts[2 * b].astype(np.float32) + parts[2 * b + 1].astype(np.float32)
        out[b] = acc.T + bias.astype(np.float32)[None, :]
    return out


def kernel(x, w_attn, b_attn, w_proj, b_proj, **run_kwargs):
    x = np.asarray(x, np.float32)
    w_attn = np.asarray(w_attn, np.float32)
    b_attn = np.asarray(b_attn, np.float32)
    w_proj = np.asarray(w_proj, np.float32)
    b_proj = np.asarray(b_proj, np.float32)

    nc = _build()
    in_maps = shard_inputs(x, w_attn, b_attn, w_proj)
    res = bass_utils.run_bass_kernel_spmd(
        nc, in_maps, core_ids=list(range(N_CORES)), **run_kwargs
    )
    parts = [r["outT"] for r in res.results]
    out = combine_outputs(parts, b_attn, w_proj, b_proj)
    kernel.last_results = res
    return out
